# revision 51
# baseline (speedup 1.0000x reference)
"""Trainium2 Bass kernel: AtomEmbeddingAndSumLastLayer (segment_reduce).

Computes: out = normalize(relu(segment_sum(relu(x @ W.T + b), batch)))
  x [1M, 118] f32, W [64, 118], b [64], batch [1M] sorted int in [0, 4096).

Strategy (8 NeuronCores, no collectives needed):
  - Atoms are cut at segment-aligned boundaries on the host so core c owns
    exactly segments [512c, 512(c+1)); per-core outputs concatenate.
  - Host pre-transposes x to xT [128, A] fp8-e4m3 with a ones-row at 118
    (folds the bias into the matmul) and zero rows above; atoms are grouped
    into 4 "superwindows" of 128 segments, each made of 4 windows of 32
    segments whose 128-atom tiles are interleaved quad-wise.
  - Device, per 128-atom tile:
      h_psum[128, 64] = xT_tile.T @ WT            (TensorE, fp8 x bf16)
      h_sb = relu(h_psum) -> bf16                 (ScalarE, chunked)
      oh[128, 32] = (iota == seg_local)           (VectorE, one op/superwin)
      s_psum[32q:32q+32, 64] += oh.T @ h_sb       (TensorE col-group q —
                                                   4 windows' seg-matmuls run
                                                   on disjoint 32-col strips)
    Epilogue per superwindow on [128, 64]: rowwise max, recip, scale, DMA.
"""

import os
import sys
import numpy as np

sys.path.insert(0, "/opt/trn_rl_repo")

import ml_dtypes  # noqa: E402

N_ATOMS = 1_000_000
D_IN = 118
K_DIM = 128  # 118 features + ones-row (bias) at 118, zero-padded to 128
ONES_ROW = D_IN
D_OUT = 64
NUM_SEG = 4096
N_CORES = 8
SEGS_PER_CORE = NUM_SEG // N_CORES  # 512
G_W = 32  # segments per window (one PE col-group)
QUAD = 4  # windows per superwindow (= PE col-groups used)
SUPER = SEGS_PER_CORE // (G_W * QUAD)  # 4 superwindows per core
P = 128
CHUNK = 16  # tiles per compute chunk (= 4 quads; 16*64 f32 = 2 psum banks)
RSPLIT = 12  # tiles per chunk relu'd on ScalarE (rest on VectorE)
XBUFS = 6
HBUFS = 4
OHBUFS = 3
PAD_ID = 200.0  # local seg id for padding atoms; never matches iota [0, G_W)

BF16 = ml_dtypes.bfloat16
FP8 = ml_dtypes.float8_e4m3

_CACHE = {}


def _build_graph(t_q: int, tmax: tuple = None, postprocess: bool = True):
    """Build the SPMD Bass graph for one core.

    t_q = padded tiles per window (multiple of QUAD); each superwindow has
    QUAD * t_q interleaved tiles.
    """
    import concourse.bass as bass
    import concourse.tile as tile
    from concourse import mybir
    from contextlib import ExitStack

    sw_tiles = QUAD * t_q  # tiles per superwindow (layout stride)
    n_tiles = SUPER * sw_tiles
    a_cols = n_tiles * P
    # per-superwindow USED tiles: trailing all-empty tiles (beyond the
    # largest window of that superwindow slot, across cores) are never
    # DMA'd or computed -- the layout is unchanged, just a prefix is used
    if tmax is None:
        tmax = (t_q,) * SUPER
    n_used = [QUAD * t for t in tmax]

    nc = bass.Bass(target_bir_lowering=False)

    xt = nc.declare_dram_parameter("xt", [K_DIM, a_cols], mybir.dt.float8e4, False)
    seg = nc.declare_dram_parameter("seg", [P, n_tiles], mybir.dt.bfloat16, False)
    wt = nc.declare_dram_parameter("wt", [K_DIM, D_OUT], mybir.dt.bfloat16, False)
    iota = nc.declare_dram_parameter("iota", [P, G_W], mybir.dt.bfloat16, False)
    out = nc.declare_dram_parameter(
        "out", [SEGS_PER_CORE, D_OUT], mybir.dt.bfloat16, True
    )

    with ExitStack() as ctx:
        tc = ctx.enter_context(tile.TileContext(nc))
        consts = ctx.enter_context(tc.tile_pool(name="consts", bufs=1))
        xpool = ctx.enter_context(tc.tile_pool(name="xp", bufs=6))
        hpool_s = ctx.enter_context(tc.tile_pool(name="hps", bufs=HBUFS))
        hpool_v = ctx.enter_context(tc.tile_pool(name="hpv", bufs=HBUFS))
        ohpool = ctx.enter_context(tc.tile_pool(name="ohp", bufs=OHBUFS))
        psum_h = ctx.enter_context(tc.tile_pool(name="psh", bufs=3, space="PSUM"))
        psum_s = ctx.enter_context(tc.tile_pool(name="pss", bufs=2, space="PSUM"))
        epi = ctx.enter_context(tc.tile_pool(name="epi", bufs=2))

        wt_sb = consts.tile([K_DIM, D_OUT], mybir.dt.bfloat16)
        nc.sync.dma_start(out=wt_sb[:], in_=wt[:, :])
        iota_sb = consts.tile([P, G_W], mybir.dt.bfloat16)
        nc.sync.dma_start(out=iota_sb[:], in_=iota[:, :])
        # sw0's seg slice ships alone so the first one-hot slices can start
        # ~1.4us earlier; the rest follows after sw0's first x pieces
        seg_a = consts.tile([P, sw_tiles], mybir.dt.bfloat16)
        nc.sync.dma_start(out=seg_a[:], in_=seg[:, :sw_tiles])
        seg_b = consts.tile([P, (SUPER - 1) * sw_tiles], mybir.dt.bfloat16)

        # "touch" the consts on VectorE once so later ops don't each carry
        # multiple DMA-lane semaphore waits (walrus wait-slot limit).
        dummy_a = consts.tile([P, 1], mybir.dt.bfloat16)
        nc.vector.tensor_copy(out=dummy_a[:], in_=iota_sb[:, :1])
        dummy_b = consts.tile([P, 1], mybir.dt.bfloat16)
        nc.vector.tensor_copy(out=dummy_b[:], in_=seg_a[:, :1])
        dummy_c = consts.tile([K_DIM, 1], mybir.dt.bfloat16)
        nc.vector.tensor_copy(out=dummy_c[:], in_=wt_sb[:, :1])
        zeros_sb = consts.tile([P, P], mybir.dt.bfloat16)
        nc.vector.memset(zeros_sb[:], 0.0)
        # prewarm ScalarE's activation table during the initial x DMA
        dummy_d = consts.tile([P, 1], mybir.dt.bfloat16)
        nc.scalar.activation(
            out=dummy_d[:], in_=dummy_a[:],
            func=mybir.ActivationFunctionType.Relu,
        )

        n_chunks = sw_tiles // CHUNK

        def piece_sizes(sw):
            # piece sizes in tiles over the used prefix: small ramp pieces
            # for sw0, then ~32-tile pieces, remainder last
            rem = n_used[sw]
            sizes = []
            if sw == 0:
                sizes = [8, 8, 16]
                rem -= 32
            while rem > 32:
                sizes.append(32)
                rem -= 32
            if rem:
                sizes.append(rem)
            return sizes

        def emit_oh(sw, oh_t, part, n_parts):
            """One tile-range slice of superwindow sw's one-hot:
            oh[p, m*G_W + g] = (iota[p, g] == seg[p, base+m])."""
            m0 = part * n_used[sw] // n_parts
            m1 = (part + 1) * n_used[sw] // n_parts
            nm = m1 - m0
            o = oh_t[:, m0 * G_W : m1 * G_W]
            iota_ap = iota_sb[:]
            in0 = bass.AP(
                tensor=iota_ap.tensor, offset=iota_ap.offset,
                ap=[iota_ap.ap[0], [0, nm], iota_ap.ap[1]],
            )
            if sw == 0:
                seg_sl = seg_a[:, m0:m1]
            else:
                b0 = (sw - 1) * sw_tiles
                seg_sl = seg_b[:, b0 + m0 : b0 + m1]
            in1 = bass.AP(
                tensor=seg_sl.tensor, offset=seg_sl.offset,
                ap=[seg_sl.ap[0], seg_sl.ap[1], [0, G_W]],
            )
            nc.vector.tensor_tensor(
                out=o.rearrange("p (t g) -> p t g", g=G_W),
                in0=in0, in1=in1, op=mybir.AluOpType.is_equal,
            )

        OH_PARTS = 4
        oh_tiles = {}

        def new_oh_tile(sw):
            t = ohpool.tile([P, G_W * sw_tiles], mybir.dt.bfloat16)
            oh_tiles[sw] = t
            return t

        oh0 = new_oh_tile(0)
        for part in range(6):
            emit_oh(0, oh0, part, n_chunks)

        def _emit_epilogue(sw, s_ps):
            # max-normalize the superwindow's 128 segment rows; the DVE
            # reads the segment sums straight from PSUM (no staging copy)
            mx = epi.tile([P, 1], mybir.dt.float32)
            nc.vector.tensor_reduce(
                out=mx[:], in_=s_ps[:], axis=mybir.AxisListType.X,
                op=mybir.AluOpType.max,
            )
            rc = epi.tile([P, 1], mybir.dt.float32)
            nc.vector.reciprocal(out=rc[:], in_=mx[:])
            o_sb = epi.tile([P, D_OUT], mybir.dt.bfloat16)
            nc.vector.tensor_scalar_mul(out=o_sb[:], in0=s_ps[:], scalar1=rc[:])
            nc.sync.dma_start(out=out[sw * P : (sw + 1) * P, :], in_=o_sb[:])

        # Software-pipelined chunk loop: h-matmuls run LOOKAHEAD chunks
        # ahead of the relu + seg-matmuls so the in-order PE queue always
        # holds ready work while a chunk's relu completes (otherwise the PE
        # idles each chunk, the HAM clock gate re-throttles to 1.2 GHz, and
        # compute falls behind the DMA stream).
        LOOKAHEAD = 2
        chunks = [(sw, chv) for sw in range(SUPER) for chv in range(n_chunks)]
        n_total = len(chunks)
        h_ctx = {}
        sw_state = {}
        x_ctx = {}

        def chunk_nt(sw, chv):
            return min(CHUNK, n_used[sw] - chv * CHUNK)

        def emit_h(ci):
            sw, chv = chunks[ci]
            nt = chunk_nt(sw, chv)
            if chv == 0:
                # issue this superwindow's x pieces (graded sizes, inline on
                # the Sync queue -- same pacing as the tuned baseline)
                base_t = sw * sw_tiles
                x_pieces, starts = [], []
                off = 0
                for pidx, ptiles in enumerate(piece_sizes(sw)):
                    size = ptiles * P
                    xp_t = xpool.tile([K_DIM, size], mybir.dt.float8e4,
                                      tag=f"xs{ptiles}")
                    p0 = base_t * P + off
                    nc.sync.dma_start(out=xp_t[:], in_=xt[:, p0 : p0 + size])
                    x_pieces.append(xp_t)
                    starts.append(off)
                    off += size
                    if sw == 0 and pidx == 1:
                        # remaining seg slices ride behind the first pieces
                        nc.sync.dma_start(
                            out=seg_b[:], in_=seg[:, sw_tiles:]
                        )
                x_ctx[sw] = (x_pieces, starts)
            x_pieces, starts = x_ctx[sw]
            h_ps = psum_h.tile([P, CHUNK * D_OUT], mybir.dt.float32)
            for i in range(nt):
                t = chv * CHUNK + i
                col = t * P
                pi = max(k for k in range(len(starts)) if starts[k] <= col)
                toff = starts[pi]
                nc.tensor.matmul(
                    out=h_ps[:, i * D_OUT : (i + 1) * D_OUT],
                    lhsT=x_pieces[pi][:, col - toff : col - toff + P],
                    rhs=wt_sb[:],
                    start=True,
                    stop=True,
                )
            h_ctx[ci] = h_ps

        for ci in range(n_total + LOOKAHEAD):
            if ci < n_total:
                emit_h(ci)
            j = ci - LOOKAHEAD
            if j < 0:
                continue
            sw, chv = chunks[j]
            if chv == 0:
                s_ps = psum_s.tile([P, D_OUT], mybir.dt.float32)
                # open the accumulation group over the whole bank with a
                # zero matmul; the col-group seg-matmuls accumulate with
                # start=False
                nc.tensor.matmul(
                    out=s_ps[:],
                    lhsT=zeros_sb[:],
                    rhs=wt_sb[:],
                    start=True,
                    stop=False,
                    skip_group_check=True,
                )
                oh_win = oh_tiles.pop(sw)
                oh_next = new_oh_tile(sw + 1) if sw + 1 < SUPER else None
                sw_state[sw] = (s_ps, oh_win, oh_next)
            s_ps, oh_win, oh_next = sw_state[sw]
            h_ps = h_ctx.pop(j)
            # during sw0 the DVE also builds sw0's own one-hot slices, so
            # give it a lighter relu share there
            nt = chunk_nt(sw, chv)
            rsplit = min(14 if sw == 0 else RSPLIT, nt)
            h_s = hpool_s.tile([P, rsplit * D_OUT], mybir.dt.bfloat16,
                               tag=f"hs{rsplit}")
            nc.scalar.activation(
                out=h_s[:],
                in_=h_ps[:, : rsplit * D_OUT],
                func=mybir.ActivationFunctionType.Relu,
            )
            h_v = None
            if nt > rsplit:
                h_v = hpool_v.tile([P, (nt - rsplit) * D_OUT],
                                   mybir.dt.bfloat16, tag=f"hv{nt - rsplit}")
                nc.vector.tensor_scalar_max(
                    out=h_v[:], in0=h_ps[:, rsplit * D_OUT : nt * D_OUT],
                    scalar1=0.0,
                )
            if sw == 0 and chv + 6 < n_chunks:
                emit_oh(0, oh_win, chv + 6, n_chunks)
            # next superwindow's one-hot, one small slice per chunk so the
            # DVE queue never carries a lump that delays relu_v (seg-matmuls
            # wait on it)
            if oh_next is not None:
                emit_oh(sw + 1, oh_next, chv, n_chunks)
            # seg-matmuls: window q of the quad accumulates on PE
            # col-group q into psum partitions [32q, 32q+32)
            for i in range(nt):
                t = chv * CHUNK + i
                q = i % QUAD
                if i < rsplit:
                    rhs = h_s[:, i * D_OUT : (i + 1) * D_OUT]
                else:
                    rhs = h_v[:, (i - rsplit) * D_OUT : (i - rsplit + 1) * D_OUT]
                nc.tensor.matmul(
                    out=s_ps[G_W * q : G_W * (q + 1), :],
                    lhsT=oh_win[:, t * G_W : (t + 1) * G_W],
                    rhs=rhs,
                    start=False,
                    stop=(chv == n_chunks - 1 and i == nt - 1),
                    tile_position=(0, G_W * q),
                    skip_group_check=True,
                )
            # epilogue for sw-1 is emitted a few chunks INTO sw so its DVE
            # ops don't delay the boundary (the psum_s bank stays valid
            # until sw+1's opener, which waits on the copy)
            if chv == 2 and sw > 0:
                _emit_epilogue(sw - 1, sw_state[sw - 1][0])
            if chv == n_chunks - 1 and sw == SUPER - 1:
                _emit_epilogue(sw, s_ps)

    if postprocess:
        _split_multi_waits(nc)
    return nc


def _split_multi_waits(nc):
    """walrus allows a single embedded sync wait per compute instruction.
    Move extra waits onto same-engine NoOps inserted just before."""
    from concourse import mybir

    n = 0
    for f in nc.m.functions:
        for blk in f.blocks:
            new_insts = []
            for inst in blk.instructions:
                si = getattr(inst, "sync_info", None)
                if si is not None and si.on_wait and len(si.on_wait) > 1:
                    extras, keep = si.on_wait[:-1], si.on_wait[-1:]
                    for wsub in extras:
                        nop = mybir.InstNoOp(
                            name=f"{inst.name}_waitnop{n}",
                            sync_info=mybir.SyncInfo(on_wait=[wsub], on_update=[]),
                            bass_nofuse=True,
                            engine=inst.engine,
                        )
                        n += 1
                        new_insts.append(nop)
                    si.on_wait = keep
                new_insts.append(inst)
            blk.instructions[:] = new_insts


def _prepare_inputs(x, w_mat, b, batch):
    """Host-side sharding/layout. Returns (in_maps, t_q)."""
    x = np.asarray(x, dtype=np.float32)
    w_mat = np.asarray(w_mat, dtype=np.float32)
    b = np.asarray(b, dtype=np.float32)
    batch = np.asarray(batch).astype(np.int64)

    # window boundaries: window j (global, 32 segs) holds atoms [wb[j], wb[j+1])
    wb = np.searchsorted(batch, np.arange(0, NUM_SEG + 1, G_W))
    counts = np.diff(wb)
    t_q = int(np.ceil(counts.max() / P))
    t_q = ((t_q + QUAD - 1) // QUAD) * QUAD  # multiple of QUAD
    # per-superwindow-slot used-tile bound (max over cores and windows)
    cc = counts.reshape(N_CORES, SUPER, QUAD)
    tmax = tuple(int(np.ceil(cc[:, s, :].max() / P)) for s in range(SUPER))

    sw_tiles = QUAD * t_q
    n_tiles = SUPER * sw_tiles
    a_cols = n_tiles * P

    wt = np.zeros((K_DIM, D_OUT), dtype=BF16)
    wt[:D_IN] = w_mat.T.astype(BF16)
    wt[ONES_ROW] = b.astype(BF16)
    iota = np.broadcast_to(
        np.arange(G_W, dtype=np.float32), (P, G_W)
    ).astype(BF16)

    xb = x.astype(FP8)
    n_win_per_core = SEGS_PER_CORE // G_W  # 16
    in_maps = []
    for c in range(N_CORES):
        xt_c = np.zeros((K_DIM, a_cols), dtype=FP8)
        seg_c = np.full((n_tiles, P), PAD_ID, dtype=np.float32)
        for sw in range(SUPER):
            for q in range(QUAD):
                gw = c * n_win_per_core + sw * QUAD + q  # global window id
                a0, a1 = wb[gw], wb[gw + 1]
                cnt = a1 - a0
                loc = (batch[a0:a1] - gw * G_W).astype(np.float32)
                # tile k of this window sits at interleaved slot (k*QUAD + q)
                for k in range((cnt + P - 1) // P):
                    m = sw * sw_tiles + k * QUAD + q  # global tile index
                    s0, s1 = k * P, min((k + 1) * P, cnt)
                    nseg = s1 - s0
                    col0 = m * P
                    xt_c[:D_IN, col0 : col0 + nseg] = xb[a0 + s0 : a0 + s1].T
                    xt_c[ONES_ROW, col0 : col0 + nseg] = 1.0
                    seg_c[m, :nseg] = loc[s0:s1]
        seg_c = np.ascontiguousarray(seg_c.T).astype(BF16)
        in_maps.append({"xt": xt_c, "seg": seg_c, "wt": wt, "iota": iota})
    return in_maps, t_q, tmax


def _install_ntff_hook_shim():
    """The trimmed container's antenv lacks axon_hooks; recreate it so
    run_bass_kernel_spmd(trace=True) can profile via the axon .so."""
    import types

    if "antenv.axon_hooks" in sys.modules:
        return
    try:
        from trn_agent_boot.trn_boot import _ntff_profile_via_ctypes

        hook = _ntff_profile_via_ctypes("/opt/axon/libaxon_pjrt.so")
    except Exception:
        hook = None
    mod = types.ModuleType("antenv.axon_hooks")
    mod._hook = hook
    mod.get_axon_ntff_profile_hook = lambda: mod._hook
    mod.set_axon_ntff_profile_hook = lambda h: setattr(mod, "_hook", h)
    sys.modules["antenv.axon_hooks"] = mod


def kernel(x, W, b, batch, num_segments):
    from concourse.bass_utils import run_bass_kernel_spmd

    assert int(num_segments) == NUM_SEG
    in_maps, t_q, tmax = _prepare_inputs(x, W, b, batch)

    key = (t_q, tmax, G_W, QUAD, CHUNK, RSPLIT, XBUFS, HBUFS, OHBUFS)
    if key not in _CACHE:
        _CACHE[key] = _build_graph(t_q, tmax)
    nc = _CACHE[key]

    trace = bool(int(os.environ.get("KERNEL_TRACE", "0")))
    if trace:
        _install_ntff_hook_shim()
    res = run_bass_kernel_spmd(
        nc, in_maps, core_ids=list(range(N_CORES)), trace=trace
    )
    kernel.last_result = res
    out = np.concatenate([r["out"] for r in res.results], axis=0)
    return out.astype(np.float32)


kernel.last_result = None



# revision 52
# speedup vs baseline: 1.1286x; 1.1286x over previous
"""Trainium2 Bass kernel: AtomEmbeddingAndSumLastLayer (segment_reduce).

Computes: out = normalize(relu(segment_sum(relu(x @ W.T + b), batch)))
  x [1M, 118] f32, W [64, 118], b [64], batch [1M] sorted int in [0, 4096).

Strategy (8 NeuronCores, no collectives needed):
  - Atoms are cut at segment-aligned boundaries on the host so core c owns
    exactly segments [512c, 512(c+1)); per-core outputs concatenate.
  - Host pre-transposes x to xT [128, A] fp8-e4m3 with a ones-row at 118
    (folds the bias into the matmul) and zero rows above; atoms are grouped
    into 4 "superwindows" of 128 segments, each made of 4 windows of 32
    segments whose 128-atom tiles are interleaved quad-wise.
  - Device, per 128-atom tile:
      h_psum[128, 64] = xT_tile.T @ WT            (TensorE, fp8 x bf16)
      h_sb = relu(h_psum) -> bf16                 (ScalarE, chunked)
      oh[128, 32] = (iota == seg_local)           (VectorE, one op/superwin)
      s_psum[32q:32q+32, 64] += oh.T @ h_sb       (TensorE col-group q —
                                                   4 windows' seg-matmuls run
                                                   on disjoint 32-col strips)
    Epilogue per superwindow on [128, 64]: rowwise max, recip, scale, DMA.
"""

import os
import sys
import numpy as np

sys.path.insert(0, "/opt/trn_rl_repo")

import ml_dtypes  # noqa: E402

N_ATOMS = 1_000_000
D_IN = 118
K_DIM = 128  # 118 features + ones-row (bias) at 118, zero-padded to 128
ONES_ROW = D_IN
D_OUT = 64
NUM_SEG = 4096
N_CORES = 8
SEGS_PER_CORE = NUM_SEG // N_CORES  # 512
G_W = 32  # segments per window (one PE col-group)
QUAD = 4  # windows per superwindow (= PE col-groups used)
SUPER = SEGS_PER_CORE // (G_W * QUAD)  # 4 superwindows per core
P = 128
CHUNK = 16  # tiles per compute chunk (= 4 quads; 16*64 f32 = 2 psum banks)
RSPLIT = 12  # tiles per chunk relu'd on ScalarE (rest on VectorE)
XBUFS = 6
HBUFS = 6
OHBUFS = 3
PAD_ID = 200.0  # local seg id for padding atoms; never matches iota [0, G_W)

BF16 = ml_dtypes.bfloat16
FP8 = ml_dtypes.float8_e4m3

_CACHE = {}


def _build_graph(t_q: int, tmax: tuple = None, postprocess: bool = True):
    """Build the SPMD Bass graph for one core.

    t_q = padded tiles per window (multiple of QUAD); each superwindow has
    QUAD * t_q interleaved tiles.
    """
    import concourse.bass as bass
    import concourse.tile as tile
    from concourse import mybir
    from contextlib import ExitStack

    sw_tiles = QUAD * t_q  # tiles per superwindow (layout stride)
    n_tiles = SUPER * sw_tiles
    a_cols = n_tiles * P
    # per-superwindow USED tiles: trailing all-empty tiles (beyond the
    # largest window of that superwindow slot, across cores) are never
    # DMA'd or computed -- the layout is unchanged, just a prefix is used
    if tmax is None:
        tmax = (t_q,) * SUPER
    n_used = [QUAD * t for t in tmax]

    nc = bass.Bass(target_bir_lowering=False)

    xt = nc.declare_dram_parameter("xt", [K_DIM, a_cols], mybir.dt.float8e4, False)
    seg = nc.declare_dram_parameter("seg", [P, n_tiles], mybir.dt.bfloat16, False)
    wt = nc.declare_dram_parameter("wt", [K_DIM, D_OUT], mybir.dt.bfloat16, False)
    iota = nc.declare_dram_parameter("iota", [P, G_W], mybir.dt.bfloat16, False)
    out = nc.declare_dram_parameter(
        "out", [SEGS_PER_CORE, D_OUT], mybir.dt.bfloat16, True
    )

    with ExitStack() as ctx:
        tc = ctx.enter_context(tile.TileContext(nc))
        consts = ctx.enter_context(tc.tile_pool(name="consts", bufs=1))
        xpool = ctx.enter_context(tc.tile_pool(name="xp", bufs=6))
        hpool_s = ctx.enter_context(tc.tile_pool(name="hps", bufs=HBUFS))
        hpool_v = ctx.enter_context(tc.tile_pool(name="hpv", bufs=HBUFS))
        ohpool = ctx.enter_context(tc.tile_pool(name="ohp", bufs=OHBUFS))
        psum_h = ctx.enter_context(tc.tile_pool(name="psh", bufs=3, space="PSUM"))
        psum_s = ctx.enter_context(tc.tile_pool(name="pss", bufs=2, space="PSUM"))
        epi = ctx.enter_context(tc.tile_pool(name="epi", bufs=2))

        wt_sb = consts.tile([K_DIM, D_OUT], mybir.dt.bfloat16)
        nc.sync.dma_start(out=wt_sb[:], in_=wt[:, :])
        iota_sb = consts.tile([P, G_W], mybir.dt.bfloat16)
        nc.sync.dma_start(out=iota_sb[:], in_=iota[:, :])
        # sw0's seg slice ships alone so the first one-hot slices can start
        # ~1.4us earlier; the rest follows after sw0's first x pieces
        seg_a = consts.tile([P, sw_tiles], mybir.dt.bfloat16)
        nc.sync.dma_start(out=seg_a[:], in_=seg[:, :sw_tiles])
        seg_b = consts.tile([P, (SUPER - 1) * sw_tiles], mybir.dt.bfloat16)

        # "touch" the consts on VectorE once so later ops don't each carry
        # multiple DMA-lane semaphore waits (walrus wait-slot limit).
        dummy_a = consts.tile([P, 1], mybir.dt.bfloat16)
        nc.vector.tensor_copy(out=dummy_a[:], in_=iota_sb[:, :1])
        dummy_b = consts.tile([P, 1], mybir.dt.bfloat16)
        nc.vector.tensor_copy(out=dummy_b[:], in_=seg_a[:, :1])
        dummy_c = consts.tile([K_DIM, 1], mybir.dt.bfloat16)
        nc.vector.tensor_copy(out=dummy_c[:], in_=wt_sb[:, :1])
        zeros_sb = consts.tile([P, P], mybir.dt.bfloat16)
        nc.vector.memset(zeros_sb[:], 0.0)
        # prewarm ScalarE's activation table during the initial x DMA
        dummy_d = consts.tile([P, 1], mybir.dt.bfloat16)
        nc.scalar.activation(
            out=dummy_d[:], in_=dummy_a[:],
            func=mybir.ActivationFunctionType.Relu,
        )

        n_chunks = sw_tiles // CHUNK

        def piece_sizes(sw):
            # piece sizes in tiles over the used prefix: small ramp pieces
            # for sw0, then ~32-tile pieces, remainder last
            rem = n_used[sw]
            sizes = []
            if sw == 0:
                sizes = [8, 8, 16]
                rem -= 32
            while rem > 32:
                sizes.append(32)
                rem -= 32
            if rem:
                sizes.append(rem)
            return sizes

        def emit_oh(sw, oh_t, part, n_parts):
            """One tile-range slice of superwindow sw's one-hot:
            oh[p, m*G_W + g] = (iota[p, g] == seg[p, base+m])."""
            m0 = part * n_used[sw] // n_parts
            m1 = (part + 1) * n_used[sw] // n_parts
            nm = m1 - m0
            o = oh_t[:, m0 * G_W : m1 * G_W]
            iota_ap = iota_sb[:]
            in0 = bass.AP(
                tensor=iota_ap.tensor, offset=iota_ap.offset,
                ap=[iota_ap.ap[0], [0, nm], iota_ap.ap[1]],
            )
            if sw == 0:
                seg_sl = seg_a[:, m0:m1]
            else:
                b0 = (sw - 1) * sw_tiles
                seg_sl = seg_b[:, b0 + m0 : b0 + m1]
            in1 = bass.AP(
                tensor=seg_sl.tensor, offset=seg_sl.offset,
                ap=[seg_sl.ap[0], seg_sl.ap[1], [0, G_W]],
            )
            nc.vector.tensor_tensor(
                out=o.rearrange("p (t g) -> p t g", g=G_W),
                in0=in0, in1=in1, op=mybir.AluOpType.is_equal,
            )

        OH_PARTS = 4
        oh_tiles = {}

        def new_oh_tile(sw):
            t = ohpool.tile([P, G_W * sw_tiles], mybir.dt.bfloat16)
            oh_tiles[sw] = t
            return t

        oh0 = new_oh_tile(0)
        for part in range(6):
            emit_oh(0, oh0, part, n_chunks)

        def _emit_epilogue(sw, s_ps):
            # max-normalize the superwindow's 128 segment rows; the DVE
            # reads the segment sums straight from PSUM (no staging copy)
            mx = epi.tile([P, 1], mybir.dt.float32)
            nc.vector.tensor_reduce(
                out=mx[:], in_=s_ps[:], axis=mybir.AxisListType.X,
                op=mybir.AluOpType.max,
            )
            rc = epi.tile([P, 1], mybir.dt.float32)
            nc.vector.reciprocal(out=rc[:], in_=mx[:])
            o_sb = epi.tile([P, D_OUT], mybir.dt.bfloat16)
            nc.vector.tensor_scalar_mul(out=o_sb[:], in0=s_ps[:], scalar1=rc[:])
            nc.sync.dma_start(out=out[sw * P : (sw + 1) * P, :], in_=o_sb[:])

        # Software-pipelined chunk loop: h-matmuls run LOOKAHEAD chunks
        # ahead of the relu + seg-matmuls so the in-order PE queue always
        # holds ready work while a chunk's relu completes (otherwise the PE
        # idles each chunk, the HAM clock gate re-throttles to 1.2 GHz, and
        # compute falls behind the DMA stream).
        LOOKAHEAD = 2
        chunks = [(sw, chv) for sw in range(SUPER) for chv in range(n_chunks)]
        n_total = len(chunks)
        h_ctx = {}
        sw_state = {}
        x_ctx = {}

        def chunk_nt(sw, chv):
            return min(CHUNK, n_used[sw] - chv * CHUNK)

        def emit_h(ci):
            sw, chv = chunks[ci]
            nt = chunk_nt(sw, chv)
            if chv == 0:
                # issue this superwindow's x pieces (graded sizes, inline on
                # the Sync queue -- same pacing as the tuned baseline)
                base_t = sw * sw_tiles
                x_pieces, starts = [], []
                off = 0
                for pidx, ptiles in enumerate(piece_sizes(sw)):
                    size = ptiles * P
                    xp_t = xpool.tile([K_DIM, size], mybir.dt.float8e4,
                                      tag=f"xs{ptiles}")
                    p0 = base_t * P + off
                    nc.sync.dma_start(out=xp_t[:], in_=xt[:, p0 : p0 + size])
                    x_pieces.append(xp_t)
                    starts.append(off)
                    off += size
                    if sw == 0 and pidx == 1:
                        # remaining seg slices ride behind the first pieces
                        nc.sync.dma_start(
                            out=seg_b[:], in_=seg[:, sw_tiles:]
                        )
                x_ctx[sw] = (x_pieces, starts)
            x_pieces, starts = x_ctx[sw]
            h_ps = psum_h.tile([P, CHUNK * D_OUT], mybir.dt.float32)
            for i in range(nt):
                t = chv * CHUNK + i
                col = t * P
                pi = max(k for k in range(len(starts)) if starts[k] <= col)
                toff = starts[pi]
                nc.tensor.matmul(
                    out=h_ps[:, i * D_OUT : (i + 1) * D_OUT],
                    lhsT=x_pieces[pi][:, col - toff : col - toff + P],
                    rhs=wt_sb[:],
                    start=True,
                    stop=True,
                )
            h_ctx[ci] = h_ps

        for ci in range(n_total + LOOKAHEAD):
            if ci < n_total:
                emit_h(ci)
            j = ci - LOOKAHEAD
            if j < 0:
                continue
            sw, chv = chunks[j]
            if chv == 0:
                s_ps = psum_s.tile([P, D_OUT], mybir.dt.float32)
                # open the accumulation group over the whole bank with a
                # zero matmul; the col-group seg-matmuls accumulate with
                # start=False
                nc.tensor.matmul(
                    out=s_ps[:],
                    lhsT=zeros_sb[:],
                    rhs=wt_sb[:],
                    start=True,
                    stop=False,
                    skip_group_check=True,
                )
                oh_win = oh_tiles.pop(sw)
                oh_next = new_oh_tile(sw + 1) if sw + 1 < SUPER else None
                sw_state[sw] = (s_ps, oh_win, oh_next)
            s_ps, oh_win, oh_next = sw_state[sw]
            h_ps = h_ctx.pop(j)
            # during sw0 the DVE also builds sw0's own one-hot slices, so
            # give it a lighter relu share there
            nt = chunk_nt(sw, chv)
            rsplit = min(14 if sw == 0 else RSPLIT, nt)
            h_s = hpool_s.tile([P, rsplit * D_OUT], mybir.dt.bfloat16,
                               tag=f"hs{rsplit}")
            nc.scalar.activation(
                out=h_s[:],
                in_=h_ps[:, : rsplit * D_OUT],
                func=mybir.ActivationFunctionType.Relu,
            )
            h_v = None
            if nt > rsplit:
                h_v = hpool_v.tile([P, (nt - rsplit) * D_OUT],
                                   mybir.dt.bfloat16, tag=f"hv{nt - rsplit}")
                nc.vector.tensor_scalar_max(
                    out=h_v[:], in0=h_ps[:, rsplit * D_OUT : nt * D_OUT],
                    scalar1=0.0,
                )
            if sw == 0 and chv + 6 < n_chunks:
                emit_oh(0, oh_win, chv + 6, n_chunks)
            # next superwindow's one-hot, one small slice per chunk so the
            # DVE queue never carries a lump that delays relu_v (seg-matmuls
            # wait on it)
            if oh_next is not None:
                emit_oh(sw + 1, oh_next, chv, n_chunks)
            # seg-matmuls: window q of the quad accumulates on PE
            # col-group q into psum partitions [32q, 32q+32)
            for i in range(nt):
                t = chv * CHUNK + i
                q = i % QUAD
                if i < rsplit:
                    rhs = h_s[:, i * D_OUT : (i + 1) * D_OUT]
                else:
                    rhs = h_v[:, (i - rsplit) * D_OUT : (i - rsplit + 1) * D_OUT]
                nc.tensor.matmul(
                    out=s_ps[G_W * q : G_W * (q + 1), :],
                    lhsT=oh_win[:, t * G_W : (t + 1) * G_W],
                    rhs=rhs,
                    start=False,
                    stop=(chv == n_chunks - 1 and i == nt - 1),
                    tile_position=(0, G_W * q),
                    skip_group_check=True,
                )
            # epilogue for sw-1 is emitted a few chunks INTO sw so its DVE
            # ops don't delay the boundary (the psum_s bank stays valid
            # until sw+1's opener, which waits on the copy)
            if chv == 2 and sw > 0:
                _emit_epilogue(sw - 1, sw_state[sw - 1][0])
            if chv == n_chunks - 1 and sw == SUPER - 1:
                _emit_epilogue(sw, s_ps)

    if postprocess:
        _split_multi_waits(nc)
    return nc


def _split_multi_waits(nc):
    """walrus allows a single embedded sync wait per compute instruction.
    Move extra waits onto same-engine NoOps inserted just before."""
    from concourse import mybir

    n = 0
    for f in nc.m.functions:
        for blk in f.blocks:
            new_insts = []
            for inst in blk.instructions:
                si = getattr(inst, "sync_info", None)
                if si is not None and si.on_wait and len(si.on_wait) > 1:
                    extras, keep = si.on_wait[:-1], si.on_wait[-1:]
                    for wsub in extras:
                        nop = mybir.InstNoOp(
                            name=f"{inst.name}_waitnop{n}",
                            sync_info=mybir.SyncInfo(on_wait=[wsub], on_update=[]),
                            bass_nofuse=True,
                            engine=inst.engine,
                        )
                        n += 1
                        new_insts.append(nop)
                    si.on_wait = keep
                new_insts.append(inst)
            blk.instructions[:] = new_insts


def _prepare_inputs(x, w_mat, b, batch):
    """Host-side sharding/layout. Returns (in_maps, t_q)."""
    x = np.asarray(x, dtype=np.float32)
    w_mat = np.asarray(w_mat, dtype=np.float32)
    b = np.asarray(b, dtype=np.float32)
    batch = np.asarray(batch).astype(np.int64)

    # window boundaries: window j (global, 32 segs) holds atoms [wb[j], wb[j+1])
    wb = np.searchsorted(batch, np.arange(0, NUM_SEG + 1, G_W))
    counts = np.diff(wb)
    t_q = int(np.ceil(counts.max() / P))
    t_q = ((t_q + QUAD - 1) // QUAD) * QUAD  # multiple of QUAD
    # per-superwindow-slot used-tile bound (max over cores and windows)
    cc = counts.reshape(N_CORES, SUPER, QUAD)
    tmax = tuple(int(np.ceil(cc[:, s, :].max() / P)) for s in range(SUPER))

    sw_tiles = QUAD * t_q
    n_tiles = SUPER * sw_tiles
    a_cols = n_tiles * P

    wt = np.zeros((K_DIM, D_OUT), dtype=BF16)
    wt[:D_IN] = w_mat.T.astype(BF16)
    wt[ONES_ROW] = b.astype(BF16)
    iota = np.broadcast_to(
        np.arange(G_W, dtype=np.float32), (P, G_W)
    ).astype(BF16)

    xb = x.astype(FP8)
    n_win_per_core = SEGS_PER_CORE // G_W  # 16
    in_maps = []
    for c in range(N_CORES):
        xt_c = np.zeros((K_DIM, a_cols), dtype=FP8)
        seg_c = np.full((n_tiles, P), PAD_ID, dtype=np.float32)
        for sw in range(SUPER):
            for q in range(QUAD):
                gw = c * n_win_per_core + sw * QUAD + q  # global window id
                a0, a1 = wb[gw], wb[gw + 1]
                cnt = a1 - a0
                loc = (batch[a0:a1] - gw * G_W).astype(np.float32)
                # tile k of this window sits at interleaved slot (k*QUAD + q)
                for k in range((cnt + P - 1) // P):
                    m = sw * sw_tiles + k * QUAD + q  # global tile index
                    s0, s1 = k * P, min((k + 1) * P, cnt)
                    nseg = s1 - s0
                    col0 = m * P
                    xt_c[:D_IN, col0 : col0 + nseg] = xb[a0 + s0 : a0 + s1].T
                    xt_c[ONES_ROW, col0 : col0 + nseg] = 1.0
                    seg_c[m, :nseg] = loc[s0:s1]
        seg_c = np.ascontiguousarray(seg_c.T).astype(BF16)
        in_maps.append({"xt": xt_c, "seg": seg_c, "wt": wt, "iota": iota})
    return in_maps, t_q, tmax


def _install_ntff_hook_shim():
    """The trimmed container's antenv lacks axon_hooks; recreate it so
    run_bass_kernel_spmd(trace=True) can profile via the axon .so."""
    import types

    if "antenv.axon_hooks" in sys.modules:
        return
    try:
        from trn_agent_boot.trn_boot import _ntff_profile_via_ctypes

        hook = _ntff_profile_via_ctypes("/opt/axon/libaxon_pjrt.so")
    except Exception:
        hook = None
    mod = types.ModuleType("antenv.axon_hooks")
    mod._hook = hook
    mod.get_axon_ntff_profile_hook = lambda: mod._hook
    mod.set_axon_ntff_profile_hook = lambda h: setattr(mod, "_hook", h)
    sys.modules["antenv.axon_hooks"] = mod


def kernel(x, W, b, batch, num_segments):
    from concourse.bass_utils import run_bass_kernel_spmd

    assert int(num_segments) == NUM_SEG
    in_maps, t_q, tmax = _prepare_inputs(x, W, b, batch)

    key = (t_q, tmax, G_W, QUAD, CHUNK, RSPLIT, XBUFS, HBUFS, OHBUFS)
    if key not in _CACHE:
        _CACHE[key] = _build_graph(t_q, tmax)
    nc = _CACHE[key]

    trace = bool(int(os.environ.get("KERNEL_TRACE", "0")))
    if trace:
        _install_ntff_hook_shim()
    res = run_bass_kernel_spmd(
        nc, in_maps, core_ids=list(range(N_CORES)), trace=trace
    )
    kernel.last_result = res
    out = np.concatenate([r["out"] for r in res.results], axis=0)
    return out.astype(np.float32)


kernel.last_result = None



# revision 53
# speedup vs baseline: 1.1460x; 1.0154x over previous
"""Trainium2 Bass kernel: AtomEmbeddingAndSumLastLayer (segment_reduce).

Computes: out = normalize(relu(segment_sum(relu(x @ W.T + b), batch)))
  x [1M, 118] f32, W [64, 118], b [64], batch [1M] sorted int in [0, 4096).

Strategy (8 NeuronCores, no collectives needed):
  - Atoms are cut at segment-aligned boundaries on the host so core c owns
    exactly segments [512c, 512(c+1)); per-core outputs concatenate.
  - Host pre-transposes x to xT [128, A] fp8-e4m3 with a ones-row at 118
    (folds the bias into the matmul) and zero rows above; atoms are grouped
    into 4 "superwindows" of 128 segments, each made of 4 windows of 32
    segments whose 128-atom tiles are interleaved quad-wise.
  - Device, per 128-atom tile:
      h_psum[128, 64] = xT_tile.T @ WT            (TensorE, fp8 x bf16)
      h_sb = relu(h_psum) -> bf16                 (ScalarE, chunked)
      oh[128, 32] = (iota == seg_local)           (VectorE, one op/superwin)
      s_psum[32q:32q+32, 64] += oh.T @ h_sb       (TensorE col-group q —
                                                   4 windows' seg-matmuls run
                                                   on disjoint 32-col strips)
    Epilogue per superwindow on [128, 64]: rowwise max, recip, scale, DMA.
"""

import os
import sys
import numpy as np

sys.path.insert(0, "/opt/trn_rl_repo")

import ml_dtypes  # noqa: E402

N_ATOMS = 1_000_000
D_IN = 118
K_DIM = 128  # 118 features + ones-row (bias) at 118, zero-padded to 128
ONES_ROW = D_IN
D_OUT = 64
NUM_SEG = 4096
N_CORES = 8
SEGS_PER_CORE = NUM_SEG // N_CORES  # 512
G_W = 32  # segments per window (one PE col-group)
QUAD = 4  # windows per superwindow (= PE col-groups used)
SUPER = SEGS_PER_CORE // (G_W * QUAD)  # 4 superwindows per core
P = 128
CHUNK = 16  # tiles per compute chunk (= 4 quads; 16*64 f32 = 2 psum banks)
RSPLIT = 12  # tiles per chunk relu'd on ScalarE (rest on VectorE)
XBUFS = 6
HBUFS = 6
OHBUFS = 3
PAD_ID = 200.0  # local seg id for padding atoms; never matches iota [0, G_W)

BF16 = ml_dtypes.bfloat16
FP8 = ml_dtypes.float8_e4m3

_CACHE = {}


def _build_graph(t_q: int, tmax: tuple = None, postprocess: bool = True):
    """Build the SPMD Bass graph for one core.

    t_q = padded tiles per window (multiple of QUAD); each superwindow has
    QUAD * t_q interleaved tiles.
    """
    import concourse.bass as bass
    import concourse.tile as tile
    from concourse import mybir
    from contextlib import ExitStack

    sw_tiles = QUAD * t_q  # tiles per superwindow (layout stride)
    n_tiles = SUPER * sw_tiles
    a_cols = n_tiles * P
    # per-superwindow USED tiles: trailing all-empty tiles (beyond the
    # largest window of that superwindow slot, across cores) are never
    # DMA'd or computed -- the layout is unchanged, just a prefix is used
    if tmax is None:
        tmax = (t_q,) * SUPER
    n_used = [QUAD * t for t in tmax]

    nc = bass.Bass(target_bir_lowering=False)

    xt = nc.declare_dram_parameter("xt", [K_DIM, a_cols], mybir.dt.float8e4, False)
    seg = nc.declare_dram_parameter("seg", [P, n_tiles], mybir.dt.bfloat16, False)
    wt = nc.declare_dram_parameter("wt", [K_DIM, D_OUT], mybir.dt.bfloat16, False)
    iota = nc.declare_dram_parameter("iota", [P, G_W], mybir.dt.bfloat16, False)
    out = nc.declare_dram_parameter(
        "out", [SEGS_PER_CORE, D_OUT], mybir.dt.bfloat16, True
    )

    with ExitStack() as ctx:
        tc = ctx.enter_context(tile.TileContext(nc))
        consts = ctx.enter_context(tc.tile_pool(name="consts", bufs=1))
        xpool = ctx.enter_context(tc.tile_pool(name="xp", bufs=6))
        hpool_s = ctx.enter_context(tc.tile_pool(name="hps", bufs=HBUFS))
        hpool_v = ctx.enter_context(tc.tile_pool(name="hpv", bufs=HBUFS))
        ohpool = ctx.enter_context(tc.tile_pool(name="ohp", bufs=OHBUFS))
        psum_h = ctx.enter_context(tc.tile_pool(name="psh", bufs=3, space="PSUM"))
        psum_s = ctx.enter_context(tc.tile_pool(name="pss", bufs=2, space="PSUM"))
        epi = ctx.enter_context(tc.tile_pool(name="epi", bufs=2))

        # tiny consts ride the idle GpSimd queue; the Sync queue starts
        # with x piece0 so compute's first data lands as early as possible
        wt_sb = consts.tile([K_DIM, D_OUT], mybir.dt.bfloat16)
        nc.gpsimd.dma_start(out=wt_sb[:], in_=wt[:, :])
        iota_sb = consts.tile([P, G_W], mybir.dt.bfloat16)
        nc.gpsimd.dma_start(out=iota_sb[:], in_=iota[:, :])
        # sw0's seg slice ships right after piece0 so the first one-hot
        # slices start early; the rest follows after sw0's first pieces
        seg_a = consts.tile([P, sw_tiles], mybir.dt.bfloat16)
        seg_b = consts.tile([P, (SUPER - 1) * sw_tiles], mybir.dt.bfloat16)

        # "touch" the consts on VectorE once so later ops don't each carry
        # multiple DMA-lane semaphore waits (walrus wait-slot limit).
        dummy_a = consts.tile([P, 1], mybir.dt.bfloat16)
        nc.vector.tensor_copy(out=dummy_a[:], in_=iota_sb[:, :1])
        dummy_b = consts.tile([P, 1], mybir.dt.bfloat16)
        nc.vector.tensor_copy(out=dummy_b[:], in_=seg_a[:, :1])
        dummy_c = consts.tile([K_DIM, 1], mybir.dt.bfloat16)
        nc.vector.tensor_copy(out=dummy_c[:], in_=wt_sb[:, :1])
        zeros_sb = consts.tile([P, P], mybir.dt.bfloat16)
        nc.vector.memset(zeros_sb[:], 0.0)
        # prewarm ScalarE's activation table during the initial x DMA
        dummy_d = consts.tile([P, 1], mybir.dt.bfloat16)
        nc.scalar.activation(
            out=dummy_d[:], in_=dummy_a[:],
            func=mybir.ActivationFunctionType.Relu,
        )

        n_chunks = sw_tiles // CHUNK

        def piece_sizes(sw):
            # piece sizes in tiles over the used prefix: small ramp pieces
            # for sw0, then ~32-tile pieces, remainder last
            rem = n_used[sw]
            sizes = []
            if sw == 0:
                sizes = [8, 8, 16]
                rem -= 32
            while rem > 32:
                sizes.append(32)
                rem -= 32
            if rem:
                sizes.append(rem)
            return sizes

        def emit_oh(sw, oh_t, part, n_parts):
            """One tile-range slice of superwindow sw's one-hot:
            oh[p, m*G_W + g] = (iota[p, g] == seg[p, base+m])."""
            m0 = part * n_used[sw] // n_parts
            m1 = (part + 1) * n_used[sw] // n_parts
            nm = m1 - m0
            o = oh_t[:, m0 * G_W : m1 * G_W]
            iota_ap = iota_sb[:]
            in0 = bass.AP(
                tensor=iota_ap.tensor, offset=iota_ap.offset,
                ap=[iota_ap.ap[0], [0, nm], iota_ap.ap[1]],
            )
            if sw == 0:
                seg_sl = seg_a[:, m0:m1]
            else:
                b0 = (sw - 1) * sw_tiles
                seg_sl = seg_b[:, b0 + m0 : b0 + m1]
            in1 = bass.AP(
                tensor=seg_sl.tensor, offset=seg_sl.offset,
                ap=[seg_sl.ap[0], seg_sl.ap[1], [0, G_W]],
            )
            nc.vector.tensor_tensor(
                out=o.rearrange("p (t g) -> p t g", g=G_W),
                in0=in0, in1=in1, op=mybir.AluOpType.is_equal,
            )

        OH_PARTS = 4
        oh_tiles = {}

        def new_oh_tile(sw):
            t = ohpool.tile([P, G_W * sw_tiles], mybir.dt.bfloat16)
            oh_tiles[sw] = t
            return t

        oh0 = new_oh_tile(0)
        for part in range(6):
            emit_oh(0, oh0, part, n_chunks)

        def _emit_epilogue(sw, s_ps):
            # max-normalize the superwindow's 128 segment rows; the DVE
            # reads the segment sums straight from PSUM (no staging copy)
            mx = epi.tile([P, 1], mybir.dt.float32)
            nc.vector.tensor_reduce(
                out=mx[:], in_=s_ps[:], axis=mybir.AxisListType.X,
                op=mybir.AluOpType.max,
            )
            rc = epi.tile([P, 1], mybir.dt.float32)
            nc.vector.reciprocal(out=rc[:], in_=mx[:])
            o_sb = epi.tile([P, D_OUT], mybir.dt.bfloat16)
            nc.vector.tensor_scalar_mul(out=o_sb[:], in0=s_ps[:], scalar1=rc[:])
            nc.sync.dma_start(out=out[sw * P : (sw + 1) * P, :], in_=o_sb[:])

        # Software-pipelined chunk loop: h-matmuls run LOOKAHEAD chunks
        # ahead of the relu + seg-matmuls so the in-order PE queue always
        # holds ready work while a chunk's relu completes (otherwise the PE
        # idles each chunk, the HAM clock gate re-throttles to 1.2 GHz, and
        # compute falls behind the DMA stream).
        LOOKAHEAD = 2
        chunks = [(sw, chv) for sw in range(SUPER) for chv in range(n_chunks)]
        n_total = len(chunks)
        h_ctx = {}
        sw_state = {}
        x_ctx = {}

        def chunk_nt(sw, chv):
            return min(CHUNK, n_used[sw] - chv * CHUNK)

        def emit_h(ci):
            sw, chv = chunks[ci]
            nt = chunk_nt(sw, chv)
            if chv == 0:
                # issue this superwindow's x pieces (graded sizes, inline on
                # the Sync queue -- same pacing as the tuned baseline)
                base_t = sw * sw_tiles
                x_pieces, starts = [], []
                off = 0
                for pidx, ptiles in enumerate(piece_sizes(sw)):
                    size = ptiles * P
                    xp_t = xpool.tile([K_DIM, size], mybir.dt.float8e4,
                                      tag=f"xs{ptiles}")
                    p0 = base_t * P + off
                    nc.sync.dma_start(out=xp_t[:], in_=xt[:, p0 : p0 + size])
                    x_pieces.append(xp_t)
                    starts.append(off)
                    off += size
                    if sw == 0 and pidx == 0:
                        nc.sync.dma_start(out=seg_a[:], in_=seg[:, :sw_tiles])
                    if sw == 0 and pidx == 1:
                        # remaining seg slices ride behind the first pieces
                        nc.sync.dma_start(
                            out=seg_b[:], in_=seg[:, sw_tiles:]
                        )
                x_ctx[sw] = (x_pieces, starts)
            x_pieces, starts = x_ctx[sw]
            h_ps = psum_h.tile([P, CHUNK * D_OUT], mybir.dt.float32)
            for i in range(nt):
                t = chv * CHUNK + i
                col = t * P
                pi = max(k for k in range(len(starts)) if starts[k] <= col)
                toff = starts[pi]
                nc.tensor.matmul(
                    out=h_ps[:, i * D_OUT : (i + 1) * D_OUT],
                    lhsT=x_pieces[pi][:, col - toff : col - toff + P],
                    rhs=wt_sb[:],
                    start=True,
                    stop=True,
                )
            h_ctx[ci] = h_ps

        for ci in range(n_total + LOOKAHEAD):
            if ci < n_total:
                emit_h(ci)
            j = ci - LOOKAHEAD
            if j < 0:
                continue
            sw, chv = chunks[j]
            if chv == 0:
                s_ps = psum_s.tile([P, D_OUT], mybir.dt.float32)
                # open the accumulation group over the whole bank with a
                # zero matmul; the col-group seg-matmuls accumulate with
                # start=False
                nc.tensor.matmul(
                    out=s_ps[:],
                    lhsT=zeros_sb[:],
                    rhs=wt_sb[:],
                    start=True,
                    stop=False,
                    skip_group_check=True,
                )
                oh_win = oh_tiles.pop(sw)
                oh_next = new_oh_tile(sw + 1) if sw + 1 < SUPER else None
                sw_state[sw] = (s_ps, oh_win, oh_next)
            s_ps, oh_win, oh_next = sw_state[sw]
            h_ps = h_ctx.pop(j)
            # during sw0 the DVE also builds sw0's own one-hot slices, so
            # give it a lighter relu share there
            nt = chunk_nt(sw, chv)
            rsplit = min(14 if sw == 0 else RSPLIT, nt)
            h_s = hpool_s.tile([P, rsplit * D_OUT], mybir.dt.bfloat16,
                               tag=f"hs{rsplit}")
            nc.scalar.activation(
                out=h_s[:],
                in_=h_ps[:, : rsplit * D_OUT],
                func=mybir.ActivationFunctionType.Relu,
            )
            h_v = None
            if nt > rsplit:
                h_v = hpool_v.tile([P, (nt - rsplit) * D_OUT],
                                   mybir.dt.bfloat16, tag=f"hv{nt - rsplit}")
                nc.vector.tensor_scalar_max(
                    out=h_v[:], in0=h_ps[:, rsplit * D_OUT : nt * D_OUT],
                    scalar1=0.0,
                )
            if sw == 0 and chv + 6 < n_chunks:
                emit_oh(0, oh_win, chv + 6, n_chunks)
            # next superwindow's one-hot, one small slice per chunk so the
            # DVE queue never carries a lump that delays relu_v (seg-matmuls
            # wait on it)
            if oh_next is not None:
                emit_oh(sw + 1, oh_next, chv, n_chunks)
            # seg-matmuls: window q of the quad accumulates on PE
            # col-group q into psum partitions [32q, 32q+32)
            for i in range(nt):
                t = chv * CHUNK + i
                q = i % QUAD
                if i < rsplit:
                    rhs = h_s[:, i * D_OUT : (i + 1) * D_OUT]
                else:
                    rhs = h_v[:, (i - rsplit) * D_OUT : (i - rsplit + 1) * D_OUT]
                nc.tensor.matmul(
                    out=s_ps[G_W * q : G_W * (q + 1), :],
                    lhsT=oh_win[:, t * G_W : (t + 1) * G_W],
                    rhs=rhs,
                    start=False,
                    stop=(chv == n_chunks - 1 and i == nt - 1),
                    tile_position=(0, G_W * q),
                    skip_group_check=True,
                )
            # epilogue for sw-1 is emitted a few chunks INTO sw so its DVE
            # ops don't delay the boundary (the psum_s bank stays valid
            # until sw+1's opener, which waits on the copy)
            if chv == 2 and sw > 0:
                _emit_epilogue(sw - 1, sw_state[sw - 1][0])
            if chv == n_chunks - 1 and sw == SUPER - 1:
                _emit_epilogue(sw, s_ps)

    if postprocess:
        _split_multi_waits(nc)
    return nc


def _split_multi_waits(nc):
    """walrus allows a single embedded sync wait per compute instruction.
    Move extra waits onto same-engine NoOps inserted just before."""
    from concourse import mybir

    n = 0
    for f in nc.m.functions:
        for blk in f.blocks:
            new_insts = []
            for inst in blk.instructions:
                si = getattr(inst, "sync_info", None)
                if si is not None and si.on_wait and len(si.on_wait) > 1:
                    extras, keep = si.on_wait[:-1], si.on_wait[-1:]
                    for wsub in extras:
                        nop = mybir.InstNoOp(
                            name=f"{inst.name}_waitnop{n}",
                            sync_info=mybir.SyncInfo(on_wait=[wsub], on_update=[]),
                            bass_nofuse=True,
                            engine=inst.engine,
                        )
                        n += 1
                        new_insts.append(nop)
                    si.on_wait = keep
                new_insts.append(inst)
            blk.instructions[:] = new_insts


def _prepare_inputs(x, w_mat, b, batch):
    """Host-side sharding/layout. Returns (in_maps, t_q)."""
    x = np.asarray(x, dtype=np.float32)
    w_mat = np.asarray(w_mat, dtype=np.float32)
    b = np.asarray(b, dtype=np.float32)
    batch = np.asarray(batch).astype(np.int64)

    # window boundaries: window j (global, 32 segs) holds atoms [wb[j], wb[j+1])
    wb = np.searchsorted(batch, np.arange(0, NUM_SEG + 1, G_W))
    counts = np.diff(wb)
    t_q = int(np.ceil(counts.max() / P))
    t_q = ((t_q + QUAD - 1) // QUAD) * QUAD  # multiple of QUAD
    # per-superwindow-slot used-tile bound (max over cores and windows)
    cc = counts.reshape(N_CORES, SUPER, QUAD)
    tmax = tuple(int(np.ceil(cc[:, s, :].max() / P)) for s in range(SUPER))

    sw_tiles = QUAD * t_q
    n_tiles = SUPER * sw_tiles
    a_cols = n_tiles * P

    wt = np.zeros((K_DIM, D_OUT), dtype=BF16)
    wt[:D_IN] = w_mat.T.astype(BF16)
    wt[ONES_ROW] = b.astype(BF16)
    iota = np.broadcast_to(
        np.arange(G_W, dtype=np.float32), (P, G_W)
    ).astype(BF16)

    xb = x.astype(FP8)
    n_win_per_core = SEGS_PER_CORE // G_W  # 16
    in_maps = []
    for c in range(N_CORES):
        xt_c = np.zeros((K_DIM, a_cols), dtype=FP8)
        seg_c = np.full((n_tiles, P), PAD_ID, dtype=np.float32)
        for sw in range(SUPER):
            for q in range(QUAD):
                gw = c * n_win_per_core + sw * QUAD + q  # global window id
                a0, a1 = wb[gw], wb[gw + 1]
                cnt = a1 - a0
                loc = (batch[a0:a1] - gw * G_W).astype(np.float32)
                # tile k of this window sits at interleaved slot (k*QUAD + q)
                for k in range((cnt + P - 1) // P):
                    m = sw * sw_tiles + k * QUAD + q  # global tile index
                    s0, s1 = k * P, min((k + 1) * P, cnt)
                    nseg = s1 - s0
                    col0 = m * P
                    xt_c[:D_IN, col0 : col0 + nseg] = xb[a0 + s0 : a0 + s1].T
                    xt_c[ONES_ROW, col0 : col0 + nseg] = 1.0
                    seg_c[m, :nseg] = loc[s0:s1]
        seg_c = np.ascontiguousarray(seg_c.T).astype(BF16)
        in_maps.append({"xt": xt_c, "seg": seg_c, "wt": wt, "iota": iota})
    return in_maps, t_q, tmax


def _install_ntff_hook_shim():
    """The trimmed container's antenv lacks axon_hooks; recreate it so
    run_bass_kernel_spmd(trace=True) can profile via the axon .so."""
    import types

    if "antenv.axon_hooks" in sys.modules:
        return
    try:
        from trn_agent_boot.trn_boot import _ntff_profile_via_ctypes

        hook = _ntff_profile_via_ctypes("/opt/axon/libaxon_pjrt.so")
    except Exception:
        hook = None
    mod = types.ModuleType("antenv.axon_hooks")
    mod._hook = hook
    mod.get_axon_ntff_profile_hook = lambda: mod._hook
    mod.set_axon_ntff_profile_hook = lambda h: setattr(mod, "_hook", h)
    sys.modules["antenv.axon_hooks"] = mod


def kernel(x, W, b, batch, num_segments):
    from concourse.bass_utils import run_bass_kernel_spmd

    assert int(num_segments) == NUM_SEG
    in_maps, t_q, tmax = _prepare_inputs(x, W, b, batch)

    key = (t_q, tmax, G_W, QUAD, CHUNK, RSPLIT, XBUFS, HBUFS, OHBUFS)
    if key not in _CACHE:
        _CACHE[key] = _build_graph(t_q, tmax)
    nc = _CACHE[key]

    trace = bool(int(os.environ.get("KERNEL_TRACE", "0")))
    if trace:
        _install_ntff_hook_shim()
    res = run_bass_kernel_spmd(
        nc, in_maps, core_ids=list(range(N_CORES)), trace=trace
    )
    kernel.last_result = res
    out = np.concatenate([r["out"] for r in res.results], axis=0)
    return out.astype(np.float32)


kernel.last_result = None

